# revision 13
# baseline (speedup 1.0000x reference)
"""Trainium2 Bass kernel for the KA-GNN (Fourier-KAN message passing GNN).

Strategy (8 NeuronCores):
 - Nodes are padded to NP (=50176) and sharded into 8 contiguous slices
   of S (=6272, 49 tiles of 128). Each core owns its slice's h, computes
   the per-node Fourier-KAN message for its slice, and the slices are
   AllGather'ed into a full message table in DRAM once per conv layer.
 - Edges are sharded by destination node. Each core gathers its edges'
   source messages from the table with dma_gather (int16 indices ->
   table is addressed in two NP/2-row halves), builds one-hot
   destination matrices with is_equal, and scatter-adds via TensorE
   matmuls accumulated in PSUM.
 - cos/sin(k*x) are evaluated with the ScalarE Sin spline (domain
   [-pi,pi]) after range reduction: t = k*x/(2pi) + 64(+0.25 for cos)
   computed by a single replication matmul (bias via a ones-row), then
   frac = t - round(t) on VectorE (f32->i32->f32 casts round to
   nearest), then sin(2pi*frac).
 - Mean-pool + readout (128x32 -> 128x1) are done on the host.
"""
import os
import sys
from dataclasses import dataclass, field

import numpy as np

sys.path.insert(0, "/opt/trn_rl_repo")

P = 128
TWO_PI = float(2.0 * np.pi)

LAST_EXEC_NS = None


@dataclass
class Cfg:
    n_nodes: int = 50000
    in_feat: int = 64
    hidden: int = 32
    grid: int = 4
    n_graphs: int = 128
    n_conv: int = 2
    neg_slope: float = 0.01
    ncores: int = 8
    np_pad: int = 50176          # padded nodes; multiple of ncores*128
    piece_t: int = 5             # dst tiles per gather piece
    c_off: float = 64.0          # positive turn offset
    use_lrelu: bool = True       # ScalarE Lrelu (not in CoreSim)

    @property
    def S(self):
        return self.np_pad // self.ncores

    @property
    def half(self):
        return self.np_pad // 2

    @property
    def NT(self):
        return self.S // P

    @property
    def col_chunks(self):
        rem, out = self.S, []
        while rem > 0:
            w = min(512, rem)
            out.append(w)
            rem -= w
        return out


CFG = Cfg()


def _prep_edges(src, dst, cfg):
    """Shard edges by dst core, group by (dst tile, src half), pad each
    group to a multiple of 128 shared across cores (SPMD program)."""
    S, NT, HALF = cfg.S, cfg.NT, cfg.half
    core = dst // S
    t = (dst % S) // P
    q = src // HALF
    dl = (dst % P).astype(np.float32)

    key = ((core * NT + t) * 2 + q).astype(np.int64)
    counts = np.bincount(key, minlength=cfg.ncores * NT * 2).reshape(
        cfg.ncores, NT, 2)
    caps = (np.ceil(counts.max(axis=0) / P) * P).astype(np.int64)  # [NT, 2]
    caps = np.maximum(caps, P)

    pieces = [list(range(i, min(i + cfg.piece_t, NT)))
              for i in range(0, NT, cfg.piece_t)]
    group_off = np.zeros((NT, 2), dtype=np.int64)
    off = 0
    for pc in pieces:
        for q_ in (0, 1):
            for t_ in pc:
                group_off[t_, q_] = off
                off += caps[t_, q_]
    tot = off
    n_chunks = tot // P

    idx_maps, dl_maps = [], []
    for c in range(cfg.ncores):
        m = core == c
        src_c, t_c, q_c, dl_c = src[m], t[m], q[m], dl[m]
        k2 = t_c * 2 + q_c
        order = np.argsort(k2, kind="stable")
        src_s, t_s, q_s, dl_s = src_c[order], t_c[order], q_c[order], dl_c[order]
        k2s = k2[order]
        grp_counts = np.bincount(k2s, minlength=NT * 2)
        starts = np.concatenate([[0], np.cumsum(grp_counts)[:-1]])
        rank = np.arange(len(k2s)) - starts[k2s]
        pos = group_off[t_s, q_s] + rank
        idx_flat = np.zeros(tot, dtype=np.int16)
        dl_flat = np.full(tot, -1.0, dtype=np.float32)
        idx_flat[pos] = (src_s - q_s * HALF).astype(np.int16)
        dl_flat[pos] = dl_s
        idx_w = np.tile(idx_flat.reshape(-1, 16).T, (8, 1))
        dl_w = np.ascontiguousarray(dl_flat.reshape(n_chunks, P).T)
        idx_maps.append(np.ascontiguousarray(idx_w))
        dl_maps.append(dl_w)

    return caps, pieces, group_off, tot, n_chunks, idx_maps, dl_maps


def _repl_matrix(n_in, bias, grid):
    R = np.zeros((n_in + 1, n_in * grid), dtype=np.float32)
    for i in range(n_in):
        for g in range(grid):
            R[i, i * grid + g] = (g + 1) / TWO_PI
    R[n_in, :] = bias
    return R


def _build_program(cfg, caps, pieces, group_off, tot, n_chunks, phase="full"):
    import concourse.bass as bass  # noqa: F401
    import concourse.mybir as mybir
    import concourse.tile as tile
    from concourse import bacc

    f32 = mybir.dt.float32
    i32 = mybir.dt.int32
    i16 = mybir.dt.int16
    AF = mybir.ActivationFunctionType
    OP = mybir.AluOpType

    IN, HID, S, NT, HALF = cfg.in_feat, cfg.hidden, cfg.S, cfg.NT, cfg.half

    nc = bacc.Bacc("TRN2", target_bir_lowering=False, debug=False)

    xT_in = nc.dram_tensor("xT_in", [IN + 1, S], f32, kind="ExternalInput")
    iota_in = nc.dram_tensor("iota_in", [P, P], f32, kind="ExternalInput")
    idx_in = nc.dram_tensor("idx_in", [P, tot // 16], i16, kind="ExternalInput")
    dstloc_in = nc.dram_tensor("dstloc_in", [P, n_chunks], f32,
                               kind="ExternalInput")
    rin_ins = [nc.dram_tensor(n, [IN + 1, P], f32, kind="ExternalInput")
               for n in ["Rin_c_a", "Rin_c_b", "Rin_s_a", "Rin_s_b"]]
    rcv_ins = [nc.dram_tensor(n, [HID + 1, P], f32, kind="ExternalInput")
               for n in ["Rcv_c", "Rcv_s"]]
    win_ins = [nc.dram_tensor(n, [P, HID], f32, kind="ExternalInput")
               for n in ["Wic_a", "Wic_b", "Wis_a", "Wis_b"]]
    wcv_ins = [nc.dram_tensor(n, [P, HID], f32, kind="ExternalInput")
               for n in [f"Wc{k}{l}" for l in range(cfg.n_conv)
                         for k in ("c", "s")]]

    h_out = nc.dram_tensor("h_out", [HID, S], f32, kind="ExternalOutput")

    msg_slice = nc.dram_tensor("msg_slice", [S, 64], f32)
    msg_table = nc.dram_tensor("msg_table", [cfg.np_pad, 64], f32,
                               addr_space="Shared")

    kmax = 0
    for pc in pieces:
        for q in (0, 1):
            kmax = max(kmax, int(sum(caps[t_, q] for t_ in pc)) // P)

    with tile.TileContext(nc) as tc:
        with tc.tile_pool(name="const", bufs=1) as cpool, \
             tc.tile_pool(name="work", bufs=2) as wpool, \
             tc.tile_pool(name="feat", bufs=2) as fpool, \
             tc.tile_pool(name="gath", bufs=2) as gpool, \
             tc.tile_pool(name="psA", bufs=2, space="PSUM") as psA, \
             tc.tile_pool(name="psB", bufs=2, space="PSUM") as psB:

            def load_const(shape, dt, src_ap, name):
                t_ = cpool.tile(shape, dt, name=name)
                nc.sync.dma_start(out=t_[:], in_=src_ap)
                return t_

            xT = load_const([IN + 1, S], f32, xT_in[:], "xT")
            iota = load_const([P, P], f32, iota_in[:], "iota")
            idxs = load_const([P, tot // 16], i16, idx_in[:], "idxs")
            dstloc = load_const([P, n_chunks], f32, dstloc_in[:], "dstloc")
            rins = [load_const([IN + 1, P], f32, a[:], f"rin{i}")
                    for i, a in enumerate(rin_ins)]
            rcvs = [load_const([HID + 1, P], f32, a[:], f"rcv{i}")
                    for i, a in enumerate(rcv_ins)]
            wins = [load_const([P, HID], f32, a[:], f"win{i}")
                    for i, a in enumerate(win_ins)]
            wcvs = [load_const([P, HID], f32, a[:], f"wcv{i}")
                    for i, a in enumerate(wcv_ins)]

            hT = cpool.tile([HID + 1, S], f32, name="hT")
            nc.gpsimd.memset(hT[HID:HID + 1, :], 1.0)

            stg = [cpool.tile([P, 4, 64], f32, name=f"stg{i}") for i in range(2)]
            nc.gpsimd.memset(stg[0][:], 0.0)
            nc.gpsimd.memset(stg[1][:], 0.0)

            MAGIC = float(1.5 * 2 ** 23)  # fp32 add rounds to nearest int

            def trig_features(r_tile, rhs_ap, w):
                tp = psA.tile([P, w], f32, tag="tp", space="PSUM", name="tp")
                nc.tensor.matmul(out=tp[:], lhsT=r_tile[:], rhs=rhs_ap,
                                 start=True, stop=True)
                nf = wpool.tile([P, w], f32, tag="nf", name="nf")
                nc.vector.tensor_scalar(out=nf[:], in0=tp[:], scalar1=MAGIC,
                                        scalar2=-MAGIC, op0=OP.add, op1=OP.add)
                fr = wpool.tile([P, w], f32, tag="fr", name="fr")
                nc.vector.tensor_tensor(out=fr[:], in0=tp[:], in1=nf[:],
                                        op=OP.subtract)
                ft = fpool.tile([P, w], f32, tag="ft", name="ft")
                nc.scalar.activation(out=ft[:], in_=fr[:], func=AF.Sin,
                                     bias=0.0, scale=TWO_PI)
                return ft

            # ---------------- input projection ----------------
            col0 = 0
            for w in cfg.col_chunks:
                cols = slice(col0, col0 + w)
                feats = [trig_features(rins[i], xT[:, cols], w)
                         for i in range(4)]
                hp = psB.tile([HID, w], f32, tag="hp", space="PSUM", name="hp")
                for i in range(4):
                    nc.tensor.matmul(out=hp[:], lhsT=wins[i][:],
                                     rhs=feats[i][:],
                                     start=(i == 0), stop=(i == 3))
                nc.vector.tensor_copy(out=hT[0:HID, cols], in_=hp[:])
                col0 += w

            # ---------------- conv layers ----------------
            n_layers = {"inproj": 0, "msg": 1, "layer1": 1}.get(phase, cfg.n_conv)
            for l in range(n_layers):
                wc, ws = wcvs[2 * l], wcvs[2 * l + 1]
                col0 = 0
                for j, w in enumerate(cfg.col_chunks):
                    cols = slice(col0, col0 + w)
                    fc = trig_features(rcvs[0], hT[:, cols], w)
                    fs = trig_features(rcvs[1], hT[:, cols], w)
                    st = stg[j % 2]
                    nsub = w // P
                    for sub in range(nsub):
                        mp = psB.tile([P, HID], f32, tag="mp", space="PSUM",
                                      name="mp")
                        sl = slice(sub * P, (sub + 1) * P)
                        nc.tensor.matmul(out=mp[:], lhsT=fc[:, sl], rhs=wc[:],
                                         start=True, stop=False)
                        nc.tensor.matmul(out=mp[:], lhsT=fs[:, sl], rhs=ws[:],
                                         start=False, stop=True)
                        nc.vector.tensor_copy(out=st[:, sub, 0:HID], in_=mp[:])
                    dst_ap = msg_slice[col0:col0 + w, :].rearrange(
                        "(s p) e -> p s e", p=P)
                    nc.sync.dma_start(out=dst_ap, in_=st[:, 0:nsub, :])
                    col0 += w

                nc.gpsimd.collective_compute(
                    "AllGather",
                    mybir.AluOpType.bypass,
                    replica_groups=[list(range(cfg.ncores))],
                    ins=[msg_slice[:]],
                    outs=[msg_table[:]],
                )
                if phase in ("msg", "gather"):
                    do_gather = phase == "gather"
                    # pull a slice of the table through SBUF so the
                    # collective result is observable
                    dbg = wpool.tile([HID, P], f32, tag="z", name="dbg")
                    nc.sync.dma_start(
                        out=dbg[:],
                        in_=msg_table[0:P, 0:HID].rearrange("p e -> e p"))
                    nc.vector.tensor_copy(out=hT[0:HID, 0:P], in_=dbg[:])
                    if do_gather:
                        for pc in pieces:
                            for q in (0, 1):
                                kpq = int(sum(caps[t_, q] for t_ in pc)) // P
                                gb = gpool.tile([P, kmax, 64], f32,
                                                tag=f"gb{q}", name=f"gbg{q}")
                                c0 = int(group_off[pc[0], q])
                                nc.gpsimd.dma_gather(
                                    out_ap=gb[:, 0:kpq, :],
                                    in_ap=msg_table[q * HALF:(q + 1) * HALF, :],
                                    idxs_ap=idxs[:, c0 // 16: c0 // 16 + kpq * 8],
                                    num_idxs=kpq * P,
                                    num_idxs_reg=kpq * P,
                                    elem_size=64,
                                    single_packet=False,
                                )
                                nc.vector.tensor_copy(
                                    out=hT[0:HID, 0:64],
                                    in_=gb[0:HID, 0, 0:64])
                    continue

                for pc in pieces:
                    gbs = {}
                    for q in (0, 1):
                        kpq = int(sum(caps[t_, q] for t_ in pc)) // P
                        gb = gpool.tile([P, kmax, 64], f32, tag=f"gb{q}",
                                        name=f"gb{q}")
                        c0 = int(group_off[pc[0], q])
                        nc.gpsimd.dma_gather(
                            out_ap=gb[:, 0:kpq, :],
                            in_ap=msg_table[q * HALF:(q + 1) * HALF, :],
                            idxs_ap=idxs[:, c0 // 16: c0 // 16 + kpq * 8],
                            num_idxs=kpq * P,
                            num_idxs_reg=kpq * P,
                            elem_size=64,
                            single_packet=False,
                        )
                        gbs[q] = gb
                    for t_ in pc:
                        nk = [int(caps[t_, 0]) // P, int(caps[t_, 1]) // P]
                        total_k = nk[0] + nk[1]
                        mps = psB.tile([HID, P], f32, tag="mps", space="PSUM",
                                       name="mps")
                        done = 0
                        for q in (0, 1):
                            base = (int(group_off[t_, q]) -
                                    int(group_off[pc[0], q])) // P
                            gcol = int(group_off[t_, q]) // P
                            for k in range(nk[q]):
                                pm = wpool.tile([P, P], f32, tag="pm", name="pm")
                                nc.vector.tensor_scalar(
                                    out=pm[:], in0=iota[:],
                                    scalar1=dstloc[:, gcol + k:gcol + k + 1],
                                    scalar2=None, op0=OP.is_equal)
                                nc.tensor.matmul(
                                    out=mps[:],
                                    lhsT=gbs[q][:, base + k, 0:HID],
                                    rhs=pm[:], start=(done == 0),
                                    stop=(done == total_k - 1))
                                done += 1
                        tcols = slice(t_ * P, (t_ + 1) * P)
                        z = wpool.tile([HID, P], f32, tag="z", name="z")
                        nc.vector.tensor_tensor(out=z[:], in0=mps[:],
                                                in1=hT[0:HID, tcols],
                                                op=OP.add)
                        if cfg.use_lrelu:
                            nc.scalar.activation(
                                out=hT[0:HID, tcols], in_=z[:], func=AF.Lrelu,
                                bias=0.0, scale=1.0, alpha=cfg.neg_slope)
                        else:
                            za = wpool.tile([HID, P], f32, tag="za", name="za")
                            nc.vector.tensor_scalar(
                                out=za[:], in0=z[:], scalar1=0.0, scalar2=None,
                                op0=OP.max)
                            zb = wpool.tile([HID, P], f32, tag="zb", name="zb")
                            nc.vector.tensor_scalar(
                                out=zb[:], in0=z[:], scalar1=0.0,
                                scalar2=cfg.neg_slope,
                                op0=OP.min, op1=OP.mult)
                            nc.vector.tensor_tensor(
                                out=hT[0:HID, tcols], in0=za[:], in1=zb[:],
                                op=OP.add)

            nc.sync.dma_start(out=h_out[:], in_=hT[0:HID, :])

    nc.finalize()
    return nc


def _kan_np(x, W, bias=None):
    g = W.shape[-1]
    k = np.arange(1, g + 1, dtype=np.float32)
    arg = x[:, :, None] * k
    B = x.shape[0]
    co = np.cos(arg).reshape(B, -1)
    si = np.sin(arg).reshape(B, -1)
    out_dim = W.shape[1]
    y = co @ W[0].reshape(out_dim, -1).T + si @ W[1].reshape(out_dim, -1).T
    if bias is not None:
        y = y + bias
    return y.astype(np.float32)


def _make_in_maps(cfg, x, W_in, W_conv, idx_maps, dl_maps):
    IN, HID, G, S = cfg.in_feat, cfg.hidden, cfg.grid, cfg.S
    x_pad = np.zeros((cfg.np_pad, IN), dtype=np.float32)
    x_pad[:cfg.n_nodes] = x
    iota_np = np.ascontiguousarray(
        np.tile(np.arange(P, dtype=np.float32)[None, :], (P, 1)))

    rin_c = _repl_matrix(IN, cfg.c_off + 0.25, G)
    rin_s = _repl_matrix(IN, cfg.c_off, G)
    rcv_c = _repl_matrix(HID, cfg.c_off + 0.25, G)
    rcv_s = _repl_matrix(HID, cfg.c_off, G)
    wic = np.ascontiguousarray(W_in[0].reshape(HID, IN * G).T)
    wis = np.ascontiguousarray(W_in[1].reshape(HID, IN * G).T)

    common = {
        "iota_in": iota_np,
        "Rin_c_a": np.ascontiguousarray(rin_c[:, :P]),
        "Rin_c_b": np.ascontiguousarray(rin_c[:, P:]),
        "Rin_s_a": np.ascontiguousarray(rin_s[:, :P]),
        "Rin_s_b": np.ascontiguousarray(rin_s[:, P:]),
        "Rcv_c": rcv_c,
        "Rcv_s": rcv_s,
        "Wic_a": np.ascontiguousarray(wic[:P]),
        "Wic_b": np.ascontiguousarray(wic[P:]),
        "Wis_a": np.ascontiguousarray(wis[:P]),
        "Wis_b": np.ascontiguousarray(wis[P:]),
    }
    for l in range(cfg.n_conv):
        common[f"Wcc{l}"] = np.ascontiguousarray(
            W_conv[l % W_conv.shape[0], 0].reshape(HID, HID * G).T)
        common[f"Wcs{l}"] = np.ascontiguousarray(
            W_conv[l % W_conv.shape[0], 1].reshape(HID, HID * G).T)

    in_maps = []
    for c in range(cfg.ncores):
        xT_c = np.zeros((IN + 1, S), dtype=np.float32)
        xT_c[:IN] = x_pad[c * S:(c + 1) * S].T
        xT_c[IN] = 1.0
        in_maps.append({**common, "xT_in": xT_c,
                        "idx_in": idx_maps[c], "dstloc_in": dl_maps[c]})
    return in_maps


def _run_spmd(nc, in_maps, ncores, bench_iters=0):
    """Execute the finalized Bass program via axon PJRT. Forked from
    concourse.bass2jax.run_bass_via_pjrt, with an optional benchmark
    loop that feeds each call's outputs back as the next call's donated
    output buffers (no host traffic per iteration)."""
    import time

    import jax
    from jax.experimental.shard_map import shard_map
    from jax.sharding import Mesh, NamedSharding, PartitionSpec

    import concourse.mybir as mybir
    from concourse import bass2jax

    bass2jax.install_neuronx_cc_hook()
    partition_name = (nc.partition_id_tensor.name
                      if nc.partition_id_tensor else None)

    in_names, out_names, out_avals, zero_outs = [], [], [], []
    for alloc in nc.m.functions[0].allocations:
        if not isinstance(alloc, mybir.MemoryLocationSet):
            continue
        name = alloc.memorylocations[0].name
        if alloc.kind == "ExternalInput":
            if name != partition_name:
                in_names.append(name)
        elif alloc.kind == "ExternalOutput":
            shape = tuple(alloc.tensor_shape)
            dtype = mybir.dt.np(alloc.dtype)
            out_names.append(name)
            out_avals.append(jax.core.ShapedArray(shape, dtype))
            zero_outs.append(np.zeros(shape, dtype))
    n_params = len(in_names)
    n_outs = len(out_avals)
    all_in_names = list(in_names) + out_names
    if partition_name is not None:
        all_in_names.append(partition_name)

    donate = tuple(range(n_params, n_params + n_outs))

    def _body(*args):
        operands = list(args)
        if partition_name is not None:
            operands.append(bass2jax.partition_id_tensor())
        outs = bass2jax._bass_exec_p.bind(
            *operands,
            out_avals=tuple(out_avals),
            in_names=tuple(all_in_names),
            out_names=tuple(out_names),
            lowering_input_output_aliases=(),
            sim_require_finite=True,
            sim_require_nnan=True,
            nc=nc,
        )
        return tuple(outs)

    devices = jax.devices()[:ncores]
    mesh = Mesh(np.asarray(devices), ("core",))
    in_specs = (PartitionSpec("core"),) * (n_params + n_outs)
    out_specs = (PartitionSpec("core"),) * n_outs
    sharded = jax.jit(
        shard_map(_body, mesh=mesh, in_specs=in_specs, out_specs=out_specs,
                  check_rep=False),
        donate_argnums=donate, keep_unused=True)

    per_core = [[np.asarray(m[name]) for name in in_names] for m in in_maps]
    concat_in = [np.concatenate([per_core[c][i] for c in range(ncores)], axis=0)
                 for i in range(n_params)]
    concat_zeros = [np.zeros((ncores * z.shape[0], *z.shape[1:]), z.dtype)
                    for z in zero_outs]

    out_arrs = sharded(*concat_in, *concat_zeros)
    results = [
        {name: np.asarray(out_arrs[i]).reshape(ncores, *out_avals[i].shape)[c]
         for i, name in enumerate(out_names)}
        for c in range(ncores)
    ]

    bench_ns = None
    if bench_iters > 0:
        sharding = NamedSharding(mesh, PartitionSpec("core"))
        dev_in = [jax.device_put(a, sharding) for a in concat_in]
        outs = sharded(*dev_in, *[np.asarray(o) for o in out_arrs])
        for o in outs:
            o.block_until_ready()
        timings = []
        for it in (2, bench_iters, bench_iters, bench_iters):
            t0 = time.perf_counter()
            for _ in range(it):
                outs = sharded(*dev_in, *outs)
            for o in outs:
                o.block_until_ready()
            t1 = time.perf_counter()
            timings.append((t1 - t0) / it)
        print(f"bench per-iter ms: {[f'{t*1e3:.3f}' for t in timings]}",
              file=sys.stderr, flush=True)
        bench_ns = int(min(timings[1:]) * 1e9)
    return results, bench_ns


def kernel(x, edge_index, batch, W_in, W_conv, W_out, b_out):
    global LAST_EXEC_NS
    cfg = CFG
    nconv_env = int(os.environ.get("KERNEL_NCONV", "0"))
    if nconv_env:
        from dataclasses import replace
        cfg = replace(cfg, n_conv=nconv_env)

    x = np.asarray(x, dtype=np.float32)
    edge_index = np.asarray(edge_index)
    batch = np.asarray(batch).astype(np.int64)
    W_in = np.asarray(W_in, dtype=np.float32)
    W_conv = np.asarray(W_conv, dtype=np.float32)
    W_out = np.asarray(W_out, dtype=np.float32)
    b_out = np.asarray(b_out, dtype=np.float32)

    src = edge_index[0].astype(np.int64)
    dst = edge_index[1].astype(np.int64)

    caps, pieces, group_off, tot, n_chunks, idx_maps, dl_maps = _prep_edges(
        src, dst, cfg)
    phase = os.environ.get("KERNEL_PHASE", "full")
    nc = _build_program(cfg, caps, pieces, group_off, tot, n_chunks, phase=phase)
    in_maps = _make_in_maps(cfg, x, W_in, W_conv, idx_maps, dl_maps)

    bench_iters = int(os.environ.get("KERNEL_BENCH_ITERS", "0"))
    results, bench_ns = _run_spmd(nc, in_maps, cfg.ncores,
                                  bench_iters=bench_iters)
    LAST_EXEC_NS = bench_ns

    h_full = np.concatenate([results[c]["h_out"] for c in range(cfg.ncores)],
                            axis=1).T[:cfg.n_nodes]

    sums = np.zeros((cfg.n_graphs, cfg.hidden), dtype=np.float64)
    np.add.at(sums, batch, h_full.astype(np.float64))
    cnt = np.bincount(batch, minlength=cfg.n_graphs).astype(np.float32)
    y = (sums / np.maximum(cnt, 1.0)[:, None]).astype(np.float32)
    out = _kan_np(y, W_out, b_out)
    return (1.0 / (1.0 + np.exp(-out))).astype(np.float32)
